# revision 10
# baseline (speedup 1.0000x reference)
"""Trainium2 Bass kernel for a 12-layer GRU LM (embed -> 12x GRU -> vocab decoder).

Strategy (V1): data-parallel over the batch axis across 8 NeuronCores.
Each core processes B_local = 4 of the 32 sequences end-to-end:
  - embedding gather via indirect DMA + PE transpose to feature-major
  - per layer: bulk input-side GEMM (gi = W_ih x + b) over all timesteps,
    then a sequential 128-step scan for the recurrent part
  - full-vocab decoder GEMM, output written token-major
No collectives; the host concatenates the 8 batch shards.

All GEMMs run in bf16 (fp32 PSUM accumulation). Activations/state are fp32;
the GEMM operands are rounded to bf16.
"""

import os
import sys

sys.path.insert(0, "/opt/trn_rl_repo")

import contextlib

import numpy as np
import ml_dtypes

import concourse.bass as bass
import concourse.tile as tile
from concourse import bacc, mybir
from concourse.bass_utils import run_bass_kernel_spmd
from concourse.masks import make_identity

F32 = mybir.dt.float32
BF16 = mybir.dt.float16  # fp16: same PE speed as bf16 (FWL), 10-bit mantissa
I32 = mybir.dt.int32

# Problem shapes (hardcoded per contract)
VOCAB, H, L, T, B = 30522, 768, 12, 128, 32
N_CORES = 8
BL = B // N_CORES          # 4 sequences per core
JH = H // 128              # 6 h-chunks
G3 = 3 * H // 128          # 18 gate-chunks
NTOK = T * BL              # 512 tokens per core
VPAD = 30720               # vocab padded to multiple of 512
VCHUNK = 2048              # decoder weight chunk (SBUF)
SCAN_UNROLL = 8

_CACHE = {}


def _build():
    nc = bacc.Bacc("TRN2", target_bir_lowering=False, debug=False,
                   num_devices=N_CORES)

    emb = nc.dram_tensor("emb", [VOCAB, H], BF16, kind="ExternalInput").ap()
    ids = nc.dram_tensor("ids", [T, BL], I32, kind="ExternalInput").ap()
    wihT = nc.dram_tensor("wihT", [L, JH, 128, 3 * H], BF16, kind="ExternalInput").ap()
    whhT = nc.dram_tensor("whhT", [L, JH, 128, 3 * H], BF16, kind="ExternalInput").ap()
    wib = nc.dram_tensor("wib", [L, 1, 3 * H], BF16, kind="ExternalInput").ap()
    bhhn = nc.dram_tensor("bhhn", [L, 128, JH], F32, kind="ExternalInput").ap()
    decT = nc.dram_tensor("decT", [JH, 128, VPAD], BF16, kind="ExternalInput").ap()
    decb = nc.dram_tensor("decb", [1, VPAD], BF16, kind="ExternalInput").ap()
    out = nc.dram_tensor("out", [NTOK, VPAD], F32, kind="ExternalOutput").ap()

    with tile.TileContext(nc) as tc, contextlib.ExitStack() as ctx:
        const = ctx.enter_context(tc.tile_pool(name="const", bufs=1))
        xpool = ctx.enter_context(tc.tile_pool(name="xpool", bufs=1))
        wpool = ctx.enter_context(tc.tile_pool(name="wpool", bufs=1))
        gpool = ctx.enter_context(tc.tile_pool(name="gpool", bufs=1))
        spool = ctx.enter_context(tc.tile_pool(name="spool", bufs=1))
        dpool = ctx.enter_context(tc.tile_pool(name="dpool", bufs=2))
        stpool = ctx.enter_context(tc.tile_pool(name="stpool", bufs=3))
        epool = ctx.enter_context(tc.tile_pool(name="epool", bufs=2))
        ps = ctx.enter_context(tc.tile_pool(name="ps", bufs=2, space="PSUM"))

        # ---- constants ----
        ids_sb = const.tile([T, BL], I32)
        nc.sync.dma_start(ids_sb[:], ids[:])
        ident = const.tile([128, 128], BF16)
        make_identity(nc, ident[:])
        ones = const.tile([1, 512], BF16)
        nc.vector.memset(ones[:], 1.0)

        # activations, feature-major: [part=h%128, j, t, b]
        xA = xpool.tile([128, JH, T, BL], BF16, tag="xA")
        xB = xpool.tile([128, JH, T, BL], BF16, tag="xB")

        # ---- embedding gather + transpose into xA ----
        for b in range(BL):
            g = epool.tile([T, H], BF16, tag="egather")
            nc.gpsimd.indirect_dma_start(
                out=g[:], out_offset=None, in_=emb[:],
                in_offset=bass.IndirectOffsetOnAxis(ap=ids_sb[:, b:b + 1], axis=0),
            )
            for j in range(JH):
                tp = ps.tile([128, 128], BF16, tag="bigps")
                nc.tensor.transpose(out=tp[:], in_=g[:, j * 128:(j + 1) * 128],
                                    identity=ident[:])
                nc.vector.tensor_copy(xA[:, j, :, b], tp[:])

        # per-layer tiles
        wih_sb = wpool.tile([128, JH, 3 * H], BF16, tag="wih")
        whh_sb = wpool.tile([128, JH, 3 * H], BF16, tag="whh")
        wib_sb = wpool.tile([1, 3 * H], BF16, tag="wib")
        bhhn_sb = wpool.tile([128, JH, 1], F32, tag="bhhn")
        bhhn_bc = wpool.tile([128, JH, BL], F32, tag="bhhnbc")
        giT = gpool.tile([128, G3, T, BL], F32, tag="giT")
        # per-half state: separate tiles so the next step's PE matmuls can
        # start as soon as half 0 of h is ready (whole-tile dep granularity)
        JH2 = JH // 2
        h32h = [spool.tile([128, JH2, BL], F32, tag=f"h32_{i}", name=f"h32_{i}") for i in range(2)]
        hbfh = [spool.tile([128, JH2, BL], BF16, tag=f"hbf_{i}", name=f"hbf_{i}") for i in range(2)]
        rzh = [spool.tile([128, JH, BL], F32, tag=f"rz_{i}", name=f"rz_{i}") for i in range(2)]
        rzsh = [spool.tile([128, JH, BL], F32, tag=f"rzs_{i}", name=f"rzs_{i}") for i in range(2)]
        ghnh = [spool.tile([128, JH2, BL], F32, tag=f"ghn_{i}", name=f"ghn_{i}") for i in range(2)]
        npreh = [spool.tile([128, JH2, BL], F32, tag=f"npre_{i}", name=f"npre_{i}") for i in range(2)]
        nth = [spool.tile([128, JH2, BL], F32, tag=f"nt_{i}", name=f"nt_{i}") for i in range(2)]
        ddh = [spool.tile([128, JH2, BL], F32, tag=f"dd_{i}", name=f"dd_{i}") for i in range(2)]
        zdh = [spool.tile([128, JH2, BL], F32, tag=f"zd_{i}", name=f"zd_{i}") for i in range(2)]

        xin, xout = xA, xB
        for layer in range(L):
            # ---- load layer weights ----
            for j in range(JH):
                nc.sync.dma_start(wih_sb[:, j, :], wihT[layer, j])
                nc.sync.dma_start(whh_sb[:, j, :], whhT[layer, j])
            nc.sync.dma_start(wib_sb[:], wib[layer])
            nc.sync.dma_start(bhhn_sb[:, :, 0], bhhn[layer])
            for b in range(BL):
                nc.vector.tensor_copy(bhhn_bc[:, :, b:b + 1], bhhn_sb[:])

            # ---- bulk input-side GEMM: giT[g, t, b] = sum_j wihT_j_g^T x_j + bias
            for g in range(G3):
                pg = ps.tile([128, 512], F32, tag="bigps")
                for j in range(JH):
                    nc.tensor.matmul(pg[:], wih_sb[:, j, g * 128:(g + 1) * 128],
                                     xin[:, j, :, :], start=(j == 0), stop=False)
                nc.tensor.matmul(pg[:], wib_sb[0:1, g * 128:(g + 1) * 128],
                                 ones[0:1, :], start=False, stop=True)
                nc.vector.tensor_copy(giT[:, g, :, :], pg[:])

            # ---- recurrent scan ----
            for i in range(2):
                nc.vector.memset(h32h[i][:], 0.0)
                nc.vector.memset(hbfh[i][:], 0.0)

            def step(t):
                # gate-dim is host-permuted: g-slot order is
                # [r0 r1 r2 z0 z1 z2 n0 n1 n2 | r3 r4 r5 z3 z4 z5 n3 n4 n5]
                # half h covers h-chunks j in [3h, 3h+3) for r/z/n.
                # PSUM: one bank per half; MM phase order lets half-0 gates
                # finish early and only needs half-0 of h first.
                pgh = [ps.tile([128, 9, BL], F32, tag=f"ghps_{i}", name=f"pgh_{i}") for i in range(2)]

                def mm_phase(ghalf, jhalf):
                    for gg in range(9):
                        g = ghalf * 9 + gg
                        for jj in range(JH2):
                            j = jhalf * JH2 + jj
                            nc.tensor.matmul(pgh[ghalf][:, gg, :],
                                             whh_sb[:, j, g * 128:(g + 1) * 128],
                                             hbfh[jhalf][:, jj, :],
                                             start=(j == 0), stop=(j == JH - 1))

                for ghalf in range(2):
                    for gg in range(9):
                        g = ghalf * 9 + gg
                        for j in range(JH):
                            nc.tensor.matmul(pgh[ghalf][:, gg, :],
                                             whh_sb[:, j, g * 128:(g + 1) * 128],
                                             hbfh[j // JH2][:, j % JH2, :],
                                             start=(j == 0), stop=(j == JH - 1))

                for i in range(2):
                    g0 = 9 * i
                    nc.vector.tensor_add(rzh[i][:], pgh[i][:, 0:6, :],
                                         giT[:, g0:g0 + 6, t, :])
                    nc.scalar.activation(rzsh[i][:], rzh[i][:],
                                         mybir.ActivationFunctionType.Sigmoid)
                    nc.vector.tensor_add(ghnh[i][:], pgh[i][:, 6:9, :],
                                         bhhn_bc[:, 3 * i:3 * i + 3, :])
                    nc.vector.tensor_mul(npreh[i][:], rzsh[i][:, 0:3, :], ghnh[i][:])
                    nc.vector.tensor_add(npreh[i][:], npreh[i][:],
                                         giT[:, g0 + 6:g0 + 9, t, :])
                    nc.scalar.activation(nth[i][:], npreh[i][:],
                                         mybir.ActivationFunctionType.Tanh)
                    nc.vector.tensor_sub(ddh[i][:], h32h[i][:], nth[i][:])
                    nc.vector.tensor_mul(zdh[i][:], rzsh[i][:, 3:6, :], ddh[i][:])
                    nc.vector.tensor_add(h32h[i][:], zdh[i][:], nth[i][:])
                    nc.vector.tensor_copy(hbfh[i][:], h32h[i][:])
                    nc.vector.tensor_copy(xout[:, 3 * i:3 * i + 3, t, :], hbfh[i][:])

            with tc.For_i(0, T, SCAN_UNROLL,
                          hint_engines=(mybir.EngineType.PE,
                                        mybir.EngineType.DVE)) as t0:
                for dt in range(SCAN_UNROLL):
                    step(t0 + dt)

            xin, xout = xout, xin

        # ---- decoder ----
        nvc = VPAD // VCHUNK
        for vc in range(nvc):
            dsb = dpool.tile([128, JH, VCHUNK], BF16, tag="decw")
            for j in range(JH):
                nc.sync.dma_start(dsb[:, j, :], decT[j, :, vc * VCHUNK:(vc + 1) * VCHUNK])
            decb_c = dpool.tile([1, VCHUNK], BF16, tag="decb")
            nc.sync.dma_start(decb_c[:], decb[0:1, vc * VCHUNK:(vc + 1) * VCHUNK])
            for tc4 in range(NTOK // 128):
                for v5 in range(VCHUNK // 512):
                    off = vc * VCHUNK + v5 * 512
                    pd = ps.tile([128, 512], F32, tag="bigps")
                    for j in range(JH):
                        nc.tensor.matmul(pd[:], xin[:, j, tc4 * 32:(tc4 + 1) * 32, :],
                                         dsb[:, j, v5 * 512:(v5 + 1) * 512],
                                         start=(j == 0), stop=False)
                    nc.tensor.matmul(pd[:], ones[0:1, 0:128],
                                     decb_c[0:1, v5 * 512:(v5 + 1) * 512],
                                     start=False, stop=True)
                    stage = stpool.tile([128, 512], F32, tag="stage")
                    nc.vector.tensor_copy(stage[:], pd[:])
                    nc.sync.dma_start(out[tc4 * 128:(tc4 + 1) * 128, off:off + 512],
                                      stage[:])

    nc.compile()
    return nc


def _prep_inputs(input_ids, embedding, w_ih, w_hh, b_ih, b_hh, dec_w, dec_b):
    bf = np.float16
    emb_np = np.ascontiguousarray(embedding.astype(bf))
    # [L, JH, 128, 3H]: partition p of chunk j holds w^T[j*128+p, :] = w[:, j*128+p]
    # gate-dim permutation: [r0 r1 r2 z0 z1 z2 n0 n1 n2 | r3 r4 r5 z3 z4 z5 n3 n4 n5]
    perm = np.concatenate([
        np.arange(gt * H + (3 * half + jj) * 128, gt * H + (3 * half + jj) * 128 + 128)
        for half in range(2) for gt in range(3) for jj in range(3)])
    w_ih_p = w_ih[:, perm, :]
    w_hh_p = w_hh[:, perm, :]
    wihT_np = np.ascontiguousarray(
        w_ih_p.transpose(0, 2, 1).reshape(L, JH, 128, 3 * H).astype(bf))
    whhT_np = np.ascontiguousarray(
        w_hh_p.transpose(0, 2, 1).reshape(L, JH, 128, 3 * H).astype(bf))
    # bias row for the input-side GEMM: b_ih + [b_hh_r, b_hh_z, 0]
    wib_np = b_ih.copy()
    wib_np[:, :2 * H] += b_hh[:, :2 * H]
    wib_np = np.ascontiguousarray(wib_np[:, perm].reshape(L, 1, 3 * H).astype(bf))
    # b_hh_n arranged [128, JH]
    bhhn_np = np.ascontiguousarray(
        b_hh[:, 2 * H:].reshape(L, JH, 128).transpose(0, 2, 1).astype(np.float32))
    decT_np = np.zeros((JH, 128, VPAD), dtype=bf)
    decT_np[:, :, :VOCAB] = dec_w.T.reshape(JH, 128, VOCAB).astype(bf)
    decb_np = np.zeros((1, VPAD), dtype=bf)
    decb_np[0, :VOCAB] = dec_b.astype(bf)

    ids32 = np.asarray(input_ids).astype(np.int32)
    shared = {"emb": emb_np, "wihT": wihT_np, "whhT": whhT_np, "wib": wib_np,
              "bhhn": bhhn_np, "decT": decT_np, "decb": decb_np}
    in_maps = []
    for c in range(N_CORES):
        m = dict(shared)
        m["ids"] = np.ascontiguousarray(ids32[:, c * BL:(c + 1) * BL])
        in_maps.append(m)
    return in_maps


def kernel(input_ids, embedding, w_ih, w_hh, b_ih, b_hh, dec_w, dec_b):
    if "nc" not in _CACHE:
        _CACHE["nc"] = _build()
    nc = _CACHE["nc"]
    in_maps = _prep_inputs(input_ids, embedding, w_ih, w_hh, b_ih, b_hh,
                           dec_w, dec_b)
    res = run_bass_kernel_spmd(nc, in_maps, core_ids=list(range(N_CORES)))
    # assemble: per-core out [NTOK, VPAD], rows token-major (t*BL + b)
    full = np.empty((T, B, VOCAB), dtype=np.float32)
    for c in range(N_CORES):
        o = res.results[c]["out"][:, :VOCAB].reshape(T, BL, VOCAB)
        full[:, c * BL:(c + 1) * BL, :] = o
    return full


# revision 11
# speedup vs baseline: 1.0398x; 1.0398x over previous
"""Trainium2 Bass kernel for a 12-layer GRU LM (embed -> 12x GRU -> vocab decoder).

Strategy (V1): data-parallel over the batch axis across 8 NeuronCores.
Each core processes B_local = 4 of the 32 sequences end-to-end:
  - embedding gather via indirect DMA + PE transpose to feature-major
  - per layer: bulk input-side GEMM (gi = W_ih x + b) over all timesteps,
    then a sequential 128-step scan for the recurrent part
  - full-vocab decoder GEMM, output written token-major
No collectives; the host concatenates the 8 batch shards.

All GEMMs run in fp16 (fp32 PSUM accumulation; fp16 loads at the same PE
rate as bf16 via fast-weight-load, with 3 extra mantissa bits). Activation
state is fp32; GEMM operands are rounded to fp16.
"""

import os
import sys

sys.path.insert(0, "/opt/trn_rl_repo")

import contextlib

import numpy as np
import ml_dtypes

import concourse.bass as bass
import concourse.tile as tile
from concourse import bacc, mybir
from concourse.bass_utils import run_bass_kernel_spmd
from concourse.masks import make_identity

F32 = mybir.dt.float32
BF16 = mybir.dt.float16  # fp16: same PE speed as bf16 (FWL), 10-bit mantissa
I32 = mybir.dt.int32

# Problem shapes (hardcoded per contract)
VOCAB, H, L, T, B = 30522, 768, 12, 128, 32
N_CORES = 8
BL = B // N_CORES          # 4 sequences per core
JH = H // 128              # 6 h-chunks
G3 = 3 * H // 128          # 18 gate-chunks
NTOK = T * BL              # 512 tokens per core
VPAD = 30720               # vocab padded to multiple of 512
VCHUNK = 2048              # decoder weight chunk (SBUF)
SCAN_UNROLL = 8

_CACHE = {}


def _build():
    nc = bacc.Bacc("TRN2", target_bir_lowering=False, debug=False,
                   num_devices=N_CORES)

    emb = nc.dram_tensor("emb", [VOCAB, H], BF16, kind="ExternalInput").ap()
    ids = nc.dram_tensor("ids", [T, BL], I32, kind="ExternalInput").ap()
    wihT = nc.dram_tensor("wihT", [L, JH, 128, 3 * H], BF16, kind="ExternalInput").ap()
    whhT = nc.dram_tensor("whhT", [L, JH, 128, 3 * H], BF16, kind="ExternalInput").ap()
    wib = nc.dram_tensor("wib", [L, 1, 3 * H], BF16, kind="ExternalInput").ap()
    bhhn = nc.dram_tensor("bhhn", [L, 128, JH], F32, kind="ExternalInput").ap()
    decT = nc.dram_tensor("decT", [JH, 128, VPAD], BF16, kind="ExternalInput").ap()
    decb = nc.dram_tensor("decb", [1, VPAD], BF16, kind="ExternalInput").ap()
    out = nc.dram_tensor("out", [NTOK, VPAD], F32, kind="ExternalOutput").ap()

    with tile.TileContext(nc) as tc, contextlib.ExitStack() as ctx:
        const = ctx.enter_context(tc.tile_pool(name="const", bufs=1))
        xpool = ctx.enter_context(tc.tile_pool(name="xpool", bufs=1))
        wpool = ctx.enter_context(tc.tile_pool(name="wpool", bufs=1))
        gpool = ctx.enter_context(tc.tile_pool(name="gpool", bufs=1))
        spool = ctx.enter_context(tc.tile_pool(name="spool", bufs=1))
        dpool = ctx.enter_context(tc.tile_pool(name="dpool", bufs=2))
        stpool = ctx.enter_context(tc.tile_pool(name="stpool", bufs=3))
        epool = ctx.enter_context(tc.tile_pool(name="epool", bufs=2))
        ps = ctx.enter_context(tc.tile_pool(name="ps", bufs=2, space="PSUM"))

        # ---- constants ----
        ids_sb = const.tile([T, BL], I32)
        nc.sync.dma_start(ids_sb[:], ids[:])
        ident = const.tile([128, 128], BF16)
        make_identity(nc, ident[:])
        ones = const.tile([1, 512], BF16)
        nc.vector.memset(ones[:], 1.0)

        # activations, feature-major: [part=h%128, j, t, b]
        xA = xpool.tile([128, JH, T, BL], BF16, tag="xA")
        xB = xpool.tile([128, JH, T, BL], BF16, tag="xB")

        # ---- embedding gather + transpose into xA ----
        for b in range(BL):
            g = epool.tile([T, H], BF16, tag="egather")
            nc.gpsimd.indirect_dma_start(
                out=g[:], out_offset=None, in_=emb[:],
                in_offset=bass.IndirectOffsetOnAxis(ap=ids_sb[:, b:b + 1], axis=0),
            )
            for j in range(JH):
                tp = ps.tile([128, 128], BF16, tag="bigps")
                nc.tensor.transpose(out=tp[:], in_=g[:, j * 128:(j + 1) * 128],
                                    identity=ident[:])
                nc.vector.tensor_copy(xA[:, j, :, b], tp[:])

        # per-layer tiles
        wih_sb = wpool.tile([128, JH, 3 * H], BF16, tag="wih")
        whh_sb = wpool.tile([128, JH, 3 * H], BF16, tag="whh")
        wib_sb = wpool.tile([1, 3 * H], BF16, tag="wib")
        bhhn_sb = wpool.tile([128, JH, 1], F32, tag="bhhn")
        bhhn_bc = wpool.tile([128, JH, BL], F32, tag="bhhnbc")
        giT = gpool.tile([128, G3, T, BL], F32, tag="giT")
        h32 = spool.tile([128, JH, BL], F32, tag="h32")
        hbf = spool.tile([128, JH, BL], BF16, tag="hbf")
        rz = spool.tile([128, 2 * JH, BL], F32, tag="rz")
        rzs = spool.tile([128, 2 * JH, BL], F32, tag="rzs")
        ghn = spool.tile([128, JH, BL], F32, tag="ghn")
        npre = spool.tile([128, JH, BL], F32, tag="npre")
        nt = spool.tile([128, JH, BL], F32, tag="nt")
        dd = spool.tile([128, JH, BL], F32, tag="dd")
        zd = spool.tile([128, JH, BL], F32, tag="zd")

        xin, xout = xA, xB
        for layer in range(L):
            # ---- load layer weights ----
            for j in range(JH):
                nc.sync.dma_start(wih_sb[:, j, :], wihT[layer, j])
                nc.sync.dma_start(whh_sb[:, j, :], whhT[layer, j])
            nc.sync.dma_start(wib_sb[:], wib[layer])
            nc.sync.dma_start(bhhn_sb[:, :, 0], bhhn[layer])
            for b in range(BL):
                nc.vector.tensor_copy(bhhn_bc[:, :, b:b + 1], bhhn_sb[:])

            # ---- bulk input-side GEMM: giT[g, t, b] = sum_j wihT_j_g^T x_j + bias
            for g in range(G3):
                pg = ps.tile([128, 512], F32, tag="bigps")
                for j in range(JH):
                    nc.tensor.matmul(pg[:], wih_sb[:, j, g * 128:(g + 1) * 128],
                                     xin[:, j, :, :], start=(j == 0), stop=False)
                nc.tensor.matmul(pg[:], wib_sb[0:1, g * 128:(g + 1) * 128],
                                 ones[0:1, :], start=False, stop=True)
                nc.vector.tensor_copy(giT[:, g, :, :], pg[:])

            # ---- recurrent scan ----
            nc.vector.memset(h32[:], 0.0)
            nc.vector.memset(hbf[:], 0.0)

            def step(t):
                # r/z and n accumulate into separate PSUM banks so the DVE
                # r/z chain overlaps the PE stream of the n-gate tiles.
                pgh_rz = ps.tile([128, 2 * JH, BL], F32, tag="ghps_rz")
                pgh_n = ps.tile([128, JH, BL], F32, tag="ghps_n")
                for g in range(2 * JH):
                    for j in range(JH):
                        nc.tensor.matmul(pgh_rz[:, g, :],
                                         whh_sb[:, j, g * 128:(g + 1) * 128],
                                         hbf[:, j, :],
                                         start=(j == 0), stop=(j == JH - 1))
                for g in range(2 * JH, G3):
                    for j in range(JH):
                        nc.tensor.matmul(pgh_n[:, g - 2 * JH, :],
                                         whh_sb[:, j, g * 128:(g + 1) * 128],
                                         hbf[:, j, :],
                                         start=(j == 0), stop=(j == JH - 1))
                # r, z gates
                nc.vector.tensor_add(rz[:], pgh_rz[:], giT[:, 0:2 * JH, t, :])
                nc.scalar.activation(rzs[:], rz[:], mybir.ActivationFunctionType.Sigmoid)
                # n gate
                nc.vector.tensor_add(ghn[:], pgh_n[:], bhhn_bc[:])
                nc.vector.tensor_mul(npre[:], rzs[:, 0:JH, :], ghn[:])
                nc.vector.tensor_add(npre[:], npre[:], giT[:, 2 * JH:G3, t, :])
                nc.scalar.activation(nt[:], npre[:], mybir.ActivationFunctionType.Tanh)
                # h' = n + z*(h - n)
                nc.vector.tensor_sub(dd[:], h32[:], nt[:])
                nc.vector.tensor_mul(zd[:], rzs[:, JH:2 * JH, :], dd[:])
                nc.vector.tensor_add(h32[:], zd[:], nt[:])
                nc.vector.tensor_copy(hbf[:], h32[:])
                nc.vector.tensor_copy(xout[:, :, t, :], hbf[:])

            with tc.For_i(0, T, SCAN_UNROLL,
                          hint_engines=(mybir.EngineType.PE,
                                        mybir.EngineType.DVE)) as t0:
                for dt in range(SCAN_UNROLL):
                    step(t0 + dt)

            xin, xout = xout, xin

        # ---- decoder ----
        nvc = VPAD // VCHUNK
        for vc in range(nvc):
            dsb = dpool.tile([128, JH, VCHUNK], BF16, tag="decw")
            for j in range(JH):
                nc.sync.dma_start(dsb[:, j, :], decT[j, :, vc * VCHUNK:(vc + 1) * VCHUNK])
            decb_c = dpool.tile([1, VCHUNK], BF16, tag="decb")
            nc.sync.dma_start(decb_c[:], decb[0:1, vc * VCHUNK:(vc + 1) * VCHUNK])
            for tc4 in range(NTOK // 128):
                for v5 in range(VCHUNK // 512):
                    off = vc * VCHUNK + v5 * 512
                    pd = ps.tile([128, 512], F32, tag="bigps")
                    for j in range(JH):
                        nc.tensor.matmul(pd[:], xin[:, j, tc4 * 32:(tc4 + 1) * 32, :],
                                         dsb[:, j, v5 * 512:(v5 + 1) * 512],
                                         start=(j == 0), stop=False)
                    nc.tensor.matmul(pd[:], ones[0:1, 0:128],
                                     decb_c[0:1, v5 * 512:(v5 + 1) * 512],
                                     start=False, stop=True)
                    stage = stpool.tile([128, 512], F32, tag="stage")
                    nc.vector.tensor_copy(stage[:], pd[:])
                    nc.sync.dma_start(out[tc4 * 128:(tc4 + 1) * 128, off:off + 512],
                                      stage[:])

    nc.compile()
    return nc


def _prep_inputs(input_ids, embedding, w_ih, w_hh, b_ih, b_hh, dec_w, dec_b):
    bf = np.float16
    emb_np = np.ascontiguousarray(embedding.astype(bf))
    # [L, JH, 128, 3H]: partition p of chunk j holds w^T[j*128+p, :] = w[:, j*128+p]
    wihT_np = np.ascontiguousarray(
        w_ih.transpose(0, 2, 1).reshape(L, JH, 128, 3 * H).astype(bf))
    whhT_np = np.ascontiguousarray(
        w_hh.transpose(0, 2, 1).reshape(L, JH, 128, 3 * H).astype(bf))
    # bias row for the input-side GEMM: b_ih + [b_hh_r, b_hh_z, 0]
    wib_np = b_ih.copy()
    wib_np[:, :2 * H] += b_hh[:, :2 * H]
    wib_np = np.ascontiguousarray(wib_np.reshape(L, 1, 3 * H).astype(bf))
    # b_hh_n arranged [128, JH]
    bhhn_np = np.ascontiguousarray(
        b_hh[:, 2 * H:].reshape(L, JH, 128).transpose(0, 2, 1).astype(np.float32))
    decT_np = np.zeros((JH, 128, VPAD), dtype=bf)
    decT_np[:, :, :VOCAB] = dec_w.T.reshape(JH, 128, VOCAB).astype(bf)
    decb_np = np.zeros((1, VPAD), dtype=bf)
    decb_np[0, :VOCAB] = dec_b.astype(bf)

    ids32 = np.asarray(input_ids).astype(np.int32)
    shared = {"emb": emb_np, "wihT": wihT_np, "whhT": whhT_np, "wib": wib_np,
              "bhhn": bhhn_np, "decT": decT_np, "decb": decb_np}
    in_maps = []
    for c in range(N_CORES):
        m = dict(shared)
        m["ids"] = np.ascontiguousarray(ids32[:, c * BL:(c + 1) * BL])
        in_maps.append(m)
    return in_maps


def kernel(input_ids, embedding, w_ih, w_hh, b_ih, b_hh, dec_w, dec_b):
    if "nc" not in _CACHE:
        _CACHE["nc"] = _build()
    nc = _CACHE["nc"]
    in_maps = _prep_inputs(input_ids, embedding, w_ih, w_hh, b_ih, b_hh,
                           dec_w, dec_b)
    res = run_bass_kernel_spmd(nc, in_maps, core_ids=list(range(N_CORES)))
    # assemble: per-core out [NTOK, VPAD], rows token-major (t*BL + b)
    full = np.empty((T, B, VOCAB), dtype=np.float32)
    for c in range(N_CORES):
        o = res.results[c]["out"][:, :VOCAB].reshape(T, BL, VOCAB)
        full[:, c * BL:(c + 1) * BL, :] = o
    return full
